# revision 1
# baseline (speedup 1.0000x reference)
"""Trainium2 Bass kernel for BlockDecomposedSSMAttention.

Math: y[b,s,:] = x[b,s,:] @ B.T @ A @ C.T   (no cross-block recurrence)
 ==>  y = x @ W  with  W = B.T @ A @ C.T    (fold params into one 1024x1024
                                             matrix -> 3x fewer FLOPs on the
                                             big tensor)

Distribution over the 8 NeuronCores:
  - x is data-parallel sharded over (batch*seq): 16384 rows -> 2048 rows/core.
  - W is computed redundantly on every core (TT = A.T @ B, then W = TT.T @ C.T),
    all in float32r (1 cycle/row at N=512, same rate as bf16, ~fp32 accuracy).
    A collective-based split-W variant measured slower and noisy: the 8-core
    AllGather costs 25-40us in trigger latency + launch-skew waits, more than
    the ~67us of redundant PE work it saves. Redundant W is deterministic.
  - main:    y_shard = x_shard @ W    (f32r matmuls, N=512 moving dim)

Host-side work is layout marshalling only (shard slicing + transposes so the
contraction dim lands on SBUF partitions); every FLOP runs on the device.
"""

import os
import sys

import numpy as np

if "/opt/trn_rl_repo" not in sys.path:
    sys.path.insert(0, "/opt/trn_rl_repo")

BATCH, SEQ, D = 4, 4096, 1024
NCORES = 8
ROWS = BATCH * SEQ            # 16384
MSH = ROWS // NCORES          # 2048 rows per core
P = 128
KT = D // P                   # 8 contraction tiles
OC = 512                      # moving free-dim chunk (fp32 max; f32r fast dim)
NOC = D // OC                 # 2
MH = MSH // 2                 # x half (SBUF staging)

_CACHE: dict = {}


def _build_nc():
    import concourse.mybir as mybir
    import concourse.tile as tile
    from concourse import bacc

    f32 = mybir.dt.float32
    f32r = mybir.dt.float32r

    nc = bacc.Bacc(
        "TRN2", target_bir_lowering=False, debug=False, num_devices=NCORES
    )

    # I/O (per-core shards; layouts chosen so every matmul operand is a
    # natural [contraction-on-partitions] SBUF load)
    xt = nc.dram_tensor("xt", [P, KT, MSH], f32, kind="ExternalInput")
    a_in = nc.dram_tensor("a_in", [D, D], f32, kind="ExternalInput")
    b_in = nc.dram_tensor("b_in", [D, D], f32, kind="ExternalInput")
    ct_in = nc.dram_tensor("ct_in", [D, D], f32, kind="ExternalInput")
    y_out = nc.dram_tensor("y_out", [MSH, D], f32, kind="ExternalOutput")

    with tile.TileContext(nc) as tc:
        with (
            tc.tile_pool(name="big", bufs=1) as big,
            tc.tile_pool(name="ycopy", bufs=6) as ycopy,
            tc.tile_pool(name="psw", bufs=4, space="PSUM") as psw,
            tc.tile_pool(name="psm", bufs=4, space="PSUM") as psm,
        ):
            # ---- SBUF loads, ordered by first consumption:
            # stage A group (j, ic) consumes a-column-chunk j (all h) and
            # b h-chunks progressively, so chunk a by columns and b by h.
            a_sb, free_a = tc.tile([P, KT, D], f32r, name="a_sb")
            b_sb, free_b = tc.tile([P, KT, D], f32r, name="b_sb")
            a_re = a_in.ap().rearrange("(ho hp) a -> hp ho a", hp=P)
            b_re = b_in.ap().rearrange("(ho hp) i -> hp ho i", hp=P)
            nc.sync.dma_start(
                a_sb[:, :, 0:P], a_re[:, :, 0:P].bitcast(f32r)
            )
            # b chunked (h-pair x ic-half) in exact stage-A consumption order
            for hq in range(4):
                nc.sync.dma_start(
                    b_sb[:, hq * 2 : (hq + 1) * 2, 0:OC],
                    b_re[:, hq * 2 : (hq + 1) * 2, 0:OC].bitcast(f32r),
                )
            for j in range(1, KT):
                nc.sync.dma_start(
                    a_sb[:, :, j * P : (j + 1) * P],
                    a_re[:, :, j * P : (j + 1) * P].bitcast(f32r),
                )
            for hq in range(4):
                nc.sync.dma_start(
                    b_sb[:, hq * 2 : (hq + 1) * 2, OC:D],
                    b_re[:, hq * 2 : (hq + 1) * 2, OC:D].bitcast(f32r),
                )
            ct_sb = big.tile([P, KT, D], f32r)
            for jq in range(2):
                nc.sync.dma_start(
                    ct_sb[:, jq * 4 : (jq + 1) * 4, :],
                    ct_in.ap().rearrange("(ao ap2) o -> ap2 ao o", ap2=P)[
                        :, jq * 4 : (jq + 1) * 4, :
                    ].bitcast(f32r),
                )
            x0_sb = big.tile([P, KT, MH], f32r)
            for mq in range(2):
                nc.sync.dma_start(
                    x0_sb[:, :, mq * OC : (mq + 1) * OC],
                    xt.ap()[:, :, mq * OC : (mq + 1) * OC].bitcast(f32r),
                )

            # ---- stage A (f32r, N=512): TT = A.T @ B, full, per core ----
            tt_sb = big.tile([P, KT, D], f32r)  # [a_p, a_tile j, i]
            for ic in range(NOC):
                for j in range(KT):
                    pw = psw.tile([P, OC], f32)
                    for h in range(KT):
                        nc.tensor.matmul(
                            pw[:],
                            a_sb[:, h, j * P : (j + 1) * P],
                            b_sb[:, h, ic * OC : (ic + 1) * OC],
                            start=(h == 0),
                            stop=(h == KT - 1),
                        )
                    nc.vector.tensor_copy(
                        tt_sb[:, j, ic * OC : (ic + 1) * OC], pw[:]
                    )
            free_b()
            free_a()

            # second x half after a/b freed (SBUF headroom)
            x1_sb, _free_x1 = tc.tile([P, KT, MH], f32r, name="x1_sb")
            for mq in range(2):
                nc.sync.dma_start(
                    x1_sb[:, :, mq * OC : (mq + 1) * OC],
                    xt.ap()[:, :, MH + mq * OC : MH + (mq + 1) * OC].bitcast(f32r),
                )

            # ---- stage B (f32r): W = TT.T @ C.T, full, per core ----
            w_sb, _free_w = tc.tile([P, KT, D], f32r, name="w_sb")  # [i_p, i_tile t, o]
            for oc in range(NOC):
                for t in range(KT):
                    pw = psw.tile([P, OC], f32)
                    for j in range(KT):
                        nc.tensor.matmul(
                            pw[:],
                            tt_sb[:, j, t * P : (t + 1) * P],
                            ct_sb[:, j, oc * OC : (oc + 1) * OC],
                            start=(j == 0),
                            stop=(j == KT - 1),
                        )
                    nc.vector.tensor_copy(
                        w_sb[:, t, oc * OC : (oc + 1) * OC], pw[:]
                    )

            # ---- main loop (f32r): y_shard = x_shard @ W ----
            for oc in range(NOC):
                for mt in range(MSH // P):
                    xh = x0_sb if mt < KT else x1_sb
                    ms = (mt % KT) * P
                    pm = psm.tile([P, OC], f32)
                    for k in range(KT):
                        nc.tensor.matmul(
                            pm[:],
                            xh[:, k, ms : ms + P],
                            w_sb[:, k, oc * OC : (oc + 1) * OC],
                            start=(k == 0),
                            stop=(k == KT - 1),
                        )
                    yt = ycopy.tile([P, OC], f32)
                    nc.vector.tensor_copy(yt[:], pm[:])
                    nc.gpsimd.dma_start(
                        y_out.ap()[mt * P : (mt + 1) * P, oc * OC : (oc + 1) * OC],
                        yt[:],
                    )

            _free_w()
            _free_x1()

    nc.compile()
    return nc


def _get_nc():
    if "nc" not in _CACHE:
        _CACHE["nc"] = _build_nc()
    return _CACHE["nc"]


def _make_in_maps(x, A, B, C):
    x2 = np.ascontiguousarray(x, dtype=np.float32).reshape(ROWS, D)
    ct = np.ascontiguousarray(C.T, dtype=np.float32)
    a_full = np.ascontiguousarray(A, dtype=np.float32)
    b_full = np.ascontiguousarray(B, dtype=np.float32)
    in_maps = []
    for c in range(NCORES):
        shard = x2[c * MSH : (c + 1) * MSH]  # [MSH, D]
        # [kp, ko, m] with element (kp,ko,m) = shard[m, ko*128+kp]
        xtc = np.ascontiguousarray(shard.reshape(MSH, KT, P).transpose(2, 1, 0))
        in_maps.append({"xt": xtc, "a_in": a_full, "b_in": b_full, "ct_in": ct})
    return in_maps


def _install_ntff_hook():
    """The agent image's ``antenv`` lacks ``axon_hooks``; recreate it and
    register the ctypes-based NTFF profile hook (same as trn_boot's
    ``_ntff_profile_via_ctypes``) so ``trace=True`` yields exec_time_ns."""
    import contextlib
    import ctypes
    import types

    if "antenv.axon_hooks" in sys.modules:
        return True
    so_path = "/opt/axon/libaxon_pjrt.so"
    if not os.path.exists(so_path):
        return False
    lib = ctypes.CDLL(so_path)
    if not hasattr(lib, "axon_start_nrt_profile"):
        return False
    lib.axon_start_nrt_profile.argtypes = [
        ctypes.POINTER(ctypes.c_int64),
        ctypes.c_size_t,
    ]
    lib.axon_start_nrt_profile.restype = ctypes.c_int64
    lib.axon_stop_nrt_profile.argtypes = [ctypes.c_char_p]
    lib.axon_stop_nrt_profile.restype = ctypes.c_int64

    @contextlib.contextmanager
    def _hook(output_dir, device_ids):
        import jax

        jax.devices()
        if device_ids:
            ids = (ctypes.c_int64 * len(device_ids))(*device_ids)
            rc = lib.axon_start_nrt_profile(ids, len(device_ids))
        else:
            rc = lib.axon_start_nrt_profile(None, 0)
        if rc != 0:
            raise RuntimeError(f"axon_start_nrt_profile rc={rc}")
        try:
            yield
        finally:
            n = lib.axon_stop_nrt_profile(str(output_dir).encode())
            print(f"ntff profile: {n} file(s) written to {output_dir}")

    mod = types.ModuleType("antenv.axon_hooks")
    _state = {"hook": _hook}
    mod.set_axon_ntff_profile_hook = lambda h: _state.__setitem__("hook", h)
    mod.get_axon_ntff_profile_hook = lambda: _state["hook"]
    sys.modules["antenv.axon_hooks"] = mod
    import antenv

    antenv.axon_hooks = mod
    return True


def run(x, A, B, C, trace=False):
    """Run on hardware; returns (y_full, exec_time_ns_or_None)."""
    from concourse import bass_utils
    from concourse.bass_interp import get_hw_module

    if trace and not _install_ntff_hook():
        trace = False
    if trace:
        # upload_artifacts pushes the NEFF dir to a remote bucket; in this
        # sandbox that can fail AFTER a successful run, losing the results.
        # Degrade to the local path. (Only touches the tracing dev path.)
        if not getattr(bass_utils.upload_artifacts, "_safe", False):
            _orig_upload = bass_utils.upload_artifacts

            def _safe_upload(tmpdir):
                try:
                    return _orig_upload(tmpdir)
                except Exception as e:
                    print(f"upload_artifacts skipped ({type(e).__name__}): {e}")
                    return str(tmpdir)

            _safe_upload._safe = True
            bass_utils.upload_artifacts = _safe_upload

    nc = _get_nc()
    in_maps = _make_in_maps(x, A, B, C)

    old_m = nc.m
    nc.m = get_hw_module(nc.m)
    try:
        res = bass_utils.run_bass_kernel_spmd(
            nc, in_maps, core_ids=list(range(NCORES)), trace=trace
        )
    finally:
        nc.m = old_m

    y = np.concatenate(
        [res.results[c]["y_out"] for c in range(NCORES)], axis=0
    ).reshape(BATCH, SEQ, D)
    return y, res.exec_time_ns


def kernel(x, A, B, C):
    y, _ = run(x, A, B, C, trace=False)
    return y



# revision 2
# speedup vs baseline: 1.3787x; 1.3787x over previous
"""Trainium2 Bass kernel for BlockDecomposedSSMAttention.

Math: y[b,s,:] = x[b,s,:] @ B.T @ A @ C.T   (no cross-block recurrence)
 ==>  y = x @ W  with  W = B.T @ A @ C.T

Distribution over the 8 NeuronCores (grid = 4 row-groups x 2 col-halves):
  core c = (rg, ch):  computes y[rg*4096:(rg+1)*4096, ch*512:(ch+1)*512]
  - x rows are split 4 ways (4096 rows/core, read by 2 cores each).
  - Each core only needs W[:, ch*512:(ch+1)*512], so the W-build stages
    shrink 2x vs full-W-per-core:  T = A @ C.T[:, half]   (1024x512)
                                   W_h = B.T @ T          (1024x512)
  - All matmul operands are bf16 (same 1 cycle/row PE rate as f32r, but
    half the HBM/SBUF bytes and fast FWL weight loads); PSUM accumulates
    fp32.  y is written bf16 and upcast to fp32 on the host.

Host-side work is layout marshalling (shard slicing, transposes, dtype
casts); every MAC runs on the device.
"""

import os
import sys

import numpy as np

if "/opt/trn_rl_repo" not in sys.path:
    sys.path.insert(0, "/opt/trn_rl_repo")

import ml_dtypes

BF16 = ml_dtypes.bfloat16

BATCH, SEQ, D = 4, 4096, 1024
NCORES = 8
RG, CH = 4, 2                 # row-groups x col-halves
ROWS = BATCH * SEQ            # 16384
MSH = ROWS // RG              # 4096 rows per core
OD = D // CH                  # 512 output cols per core
P = 128
KT = D // P                   # 8 contraction tiles

_CACHE: dict = {}


def _build_nc():
    import concourse.mybir as mybir
    import concourse.tile as tile
    from concourse import bacc

    f32 = mybir.dt.float32
    bf16 = mybir.dt.bfloat16

    nc = bacc.Bacc(
        "TRN2", target_bir_lowering=False, debug=False, num_devices=NCORES
    )

    # Per-core inputs (bf16, contraction dim on partitions):
    #   at [kp, ko, j]  = A[j, ko*128+kp]            (A.T, replicated)
    #   bt [jp, jo, i]  = B[jo*128+jp, i]            (B,   replicated)
    #   ct [kp, ko, o]  = C[ch*512+o, ko*128+kp]     (C.T col-half)
    #   xt [ip, io, m]  = x2[rg*4096+m, io*128+ip]   (x row-shard, transposed)
    at_in = nc.dram_tensor("at_in", [P, KT, D], bf16, kind="ExternalInput")
    b_in = nc.dram_tensor("b_in", [P, KT, D], bf16, kind="ExternalInput")
    ct_in = nc.dram_tensor("ct_in", [P, KT, OD], bf16, kind="ExternalInput")
    xt = nc.dram_tensor("xt", [P, KT, MSH], bf16, kind="ExternalInput")
    y_out = nc.dram_tensor("y_out", [MSH, OD], bf16, kind="ExternalOutput")

    with tile.TileContext(nc) as tc:
        with (
            tc.tile_pool(name="big", bufs=1) as big,
            tc.tile_pool(name="ycopy", bufs=6) as ycopy,
            tc.tile_pool(name="pst", bufs=2, space="PSUM") as pst,
            tc.tile_pool(name="psm", bufs=4, space="PSUM") as psm,
        ):
            at_sb = big.tile([P, KT, D], bf16)
            ct_sb = big.tile([P, KT, OD], bf16)
            bt_sb = big.tile([P, KT, D], bf16)
            t_sb = big.tile([P, KT, OD], bf16)
            w_sb = big.tile([P, KT, OD], bf16)
            xt_sb = big.tile([P, KT, MSH], bf16)

            # ---- input DMAs, ordered by first consumption ----
            # stage-1 needs ct[:, kt, :] (kt ascending) and at[:, :, j-chunk]
            nc.sync.dma_start(ct_sb[:, 0:2, :], ct_in.ap()[:, 0:2, :])
            nc.sync.dma_start(
                at_sb[:, :, 0 : 2 * P], at_in.ap()[:, :, 0 : 2 * P]
            )
            nc.sync.dma_start(ct_sb[:, 2:8, :], ct_in.ap()[:, 2:8, :])
            for jq in range(1, 4):
                nc.sync.dma_start(
                    at_sb[:, :, jq * 2 * P : (jq + 1) * 2 * P],
                    at_in.ap()[:, :, jq * 2 * P : (jq + 1) * 2 * P],
                )
            for iq in range(4):
                nc.sync.dma_start(
                    bt_sb[:, :, iq * 2 * P : (iq + 1) * 2 * P],
                    b_in.ap()[:, :, iq * 2 * P : (iq + 1) * 2 * P],
                )
            for mq in range(8):
                nc.sync.dma_start(
                    xt_sb[:, :, mq * 512 : (mq + 1) * 512],
                    xt.ap()[:, :, mq * 512 : (mq + 1) * 512],
                )

            # ---- stage 1: T = A @ Ct_half  [1024 x 512] ----
            for jt in range(KT):
                ps = pst.tile([P, OD], f32)
                for kt in range(KT):
                    nc.tensor.matmul(
                        ps[:],
                        at_sb[:, kt, jt * P : (jt + 1) * P],
                        ct_sb[:, kt, :],
                        start=(kt == 0),
                        stop=(kt == KT - 1),
                    )
                nc.vector.tensor_copy(t_sb[:, jt, :], ps[:])

            # ---- stage 2: W_h = B.T @ T  [1024 x 512] ----
            for it in range(KT):
                ps = pst.tile([P, OD], f32)
                for jt in range(KT):
                    nc.tensor.matmul(
                        ps[:],
                        bt_sb[:, jt, it * P : (it + 1) * P],
                        t_sb[:, jt, :],
                        start=(jt == 0),
                        stop=(jt == KT - 1),
                    )
                nc.vector.tensor_copy(w_sb[:, it, :], ps[:])

            # ---- main: y_shard = x_shard @ W_h  [4096 x 512] ----
            for mt in range(MSH // P):
                pm = psm.tile([P, OD], f32)
                for it in range(KT):
                    nc.tensor.matmul(
                        pm[:],
                        xt_sb[:, it, mt * P : (mt + 1) * P],
                        w_sb[:, it, :],
                        start=(it == 0),
                        stop=(it == KT - 1),
                    )
                yt = ycopy.tile([P, OD], bf16)
                nc.vector.tensor_copy(yt[:], pm[:])
                nc.gpsimd.dma_start(
                    y_out.ap()[mt * P : (mt + 1) * P, :], yt[:]
                )

    nc.compile()
    return nc


def _get_nc():
    if "nc" not in _CACHE:
        _CACHE["nc"] = _build_nc()
    return _CACHE["nc"]


def _make_in_maps(x, A, B, C):
    x2 = np.ascontiguousarray(x, dtype=np.float32).reshape(ROWS, D)
    # at[kp, ko, j] = A[j, ko*128+kp]  == A.reshape(D, KT, P).transpose(2,1,0)
    at = np.ascontiguousarray(
        np.asarray(A, np.float32).reshape(D, KT, P).transpose(2, 1, 0)
    ).astype(BF16)
    # bt[jp, jo, i] = B[jo*128+jp, i]  == B.reshape(KT, P, D).transpose(1,0,2)
    bt = np.ascontiguousarray(
        np.asarray(B, np.float32).reshape(KT, P, D).transpose(1, 0, 2)
    ).astype(BF16)
    in_maps = []
    for c in range(NCORES):
        rg, ch = divmod(c, CH)
        # ct[kp, ko, o] = C[ch*512+o, ko*128+kp]
        csl = np.asarray(C, np.float32)[ch * OD : (ch + 1) * OD, :]  # [OD, D]
        ct = np.ascontiguousarray(
            csl.T.reshape(KT, P, OD).transpose(1, 0, 2)
        ).astype(BF16)
        shard = x2[rg * MSH : (rg + 1) * MSH]  # [MSH, D]
        xtc = np.ascontiguousarray(
            shard.reshape(MSH, KT, P).transpose(2, 1, 0)
        ).astype(BF16)
        in_maps.append({"at_in": at, "b_in": bt, "ct_in": ct, "xt": xtc})
    return in_maps


def _install_ntff_hook():
    """The agent image's ``antenv`` lacks ``axon_hooks``; recreate it and
    register the ctypes-based NTFF profile hook (same as trn_boot's
    ``_ntff_profile_via_ctypes``) so ``trace=True`` yields exec_time_ns."""
    import contextlib
    import ctypes
    import types

    if "antenv.axon_hooks" in sys.modules:
        return True
    so_path = "/opt/axon/libaxon_pjrt.so"
    if not os.path.exists(so_path):
        return False
    lib = ctypes.CDLL(so_path)
    if not hasattr(lib, "axon_start_nrt_profile"):
        return False
    lib.axon_start_nrt_profile.argtypes = [
        ctypes.POINTER(ctypes.c_int64),
        ctypes.c_size_t,
    ]
    lib.axon_start_nrt_profile.restype = ctypes.c_int64
    lib.axon_stop_nrt_profile.argtypes = [ctypes.c_char_p]
    lib.axon_stop_nrt_profile.restype = ctypes.c_int64

    @contextlib.contextmanager
    def _hook(output_dir, device_ids):
        import jax

        jax.devices()
        if device_ids:
            ids = (ctypes.c_int64 * len(device_ids))(*device_ids)
            rc = lib.axon_start_nrt_profile(ids, len(device_ids))
        else:
            rc = lib.axon_start_nrt_profile(None, 0)
        if rc != 0:
            raise RuntimeError(f"axon_start_nrt_profile rc={rc}")
        try:
            yield
        finally:
            n = lib.axon_stop_nrt_profile(str(output_dir).encode())
            print(f"ntff profile: {n} file(s) written to {output_dir}")

    mod = types.ModuleType("antenv.axon_hooks")
    _state = {"hook": _hook}
    mod.set_axon_ntff_profile_hook = lambda h: _state.__setitem__("hook", h)
    mod.get_axon_ntff_profile_hook = lambda: _state["hook"]
    sys.modules["antenv.axon_hooks"] = mod
    import antenv

    antenv.axon_hooks = mod
    return True


def run(x, A, B, C, trace=False):
    """Run on hardware; returns (y_full, exec_time_ns_or_None)."""
    from concourse import bass_utils
    from concourse.bass_interp import get_hw_module

    if trace and not _install_ntff_hook():
        trace = False
    if trace:
        # upload_artifacts pushes the NEFF dir to a remote bucket; in this
        # sandbox that can fail AFTER a successful run, losing the results.
        # Degrade to the local path. (Only touches the tracing dev path.)
        if not getattr(bass_utils.upload_artifacts, "_safe", False):
            _orig_upload = bass_utils.upload_artifacts

            def _safe_upload(tmpdir):
                try:
                    return _orig_upload(tmpdir)
                except Exception as e:
                    print(f"upload_artifacts skipped ({type(e).__name__}): {e}")
                    return str(tmpdir)

            _safe_upload._safe = True
            bass_utils.upload_artifacts = _safe_upload

    nc = _get_nc()
    in_maps = _make_in_maps(x, A, B, C)

    old_m = nc.m
    nc.m = get_hw_module(nc.m)
    try:
        res = bass_utils.run_bass_kernel_spmd(
            nc, in_maps, core_ids=list(range(NCORES)), trace=trace
        )
    finally:
        nc.m = old_m

    y2 = np.empty((ROWS, D), dtype=np.float32)
    for c in range(NCORES):
        rg, ch = divmod(c, CH)
        y2[rg * MSH : (rg + 1) * MSH, ch * OD : (ch + 1) * OD] = res.results[
            c
        ]["y_out"].astype(np.float32)
    return y2.reshape(BATCH, SEQ, D), res.exec_time_ns


def kernel(x, A, B, C):
    y, _ = run(x, A, B, C, trace=False)
    return y


# revision 5
# speedup vs baseline: 1.4661x; 1.0634x over previous
"""Trainium2 Bass kernel for BlockDecomposedSSMAttention.

Math: y[b,s,:] = x[b,s,:] @ B.T @ A @ C.T   (no cross-block recurrence)
 ==>  y = x @ W  with  W = B.T @ A @ C.T

Distribution over the 8 NeuronCores (grid = 2 row-groups x 4 col-quarters):
  core c = (rg, cq):  computes y[rg*8192:(rg+1)*8192, cq*256:(cq+1)*256]
  - x rows are split 2 ways (8192 rows/core, read by 4 cores each).
  - Each core only needs W[:, cq*256:(cq+1)*256], so the W-build stages
    shrink 4x vs full-W-per-core:  T = A @ C.T[:, quarter]   (1024x256)
                                   W_q = B.T @ T             (1024x256)
  - Stages run kt-outer so A.T / C.T / B stream in as fully-contiguous
    256 KiB per-k-tile chunks (2 KB descriptor rows = DMA line rate) in
    exactly consumption order.
  - Main loop keeps the moving dim at 512 by making W the stationary
    operand: psum tiles are y.T [128 o' x 512 m]; the host transposes.
  - All matmul operands are bf16 (same 1 cycle/row PE rate as f32r, half
    the HBM/SBUF bytes, fast FWL weight loads); PSUM accumulates fp32.
    y is written bf16 and upcast to fp32 on the host.

Host-side work is layout marshalling (shard slicing, transposes, dtype
casts); every MAC runs on the device.
"""

import os
import sys

import numpy as np

if "/opt/trn_rl_repo" not in sys.path:
    sys.path.insert(0, "/opt/trn_rl_repo")

import ml_dtypes

BF16 = ml_dtypes.bfloat16

BATCH, SEQ, D = 4, 4096, 1024
NCORES = 8
RG, CQ = 2, 4                 # row-groups x col-quarters
ROWS = BATCH * SEQ            # 16384
MSH = ROWS // RG              # 8192 rows per core
OD = D // CQ                  # 256 output cols per core
P = 128
KT = D // P                   # 8 contraction tiles
MC = 512                      # moving chunk of m in the main loop
NMC = MSH // MC               # 16 m-chunks
NOT = OD // P                 # 2 o'-tiles

_CACHE: dict = {}


def _build_nc():
    import concourse.mybir as mybir
    import concourse.tile as tile
    from concourse import bacc

    f32 = mybir.dt.float32
    bf16 = mybir.dt.bfloat16

    nc = bacc.Bacc(
        "TRN2", target_bir_lowering=False, debug=False, num_devices=NCORES
    )

    # Per-core inputs (bf16, contraction dim on partitions, k/j-tile-major
    # so each DMA chunk is one fully-contiguous slab):
    #   at [ko, kp, j]  = A[j, ko*128+kp]            (A.T, replicated)
    #   bt [jo, jp, i]  = B[jo*128+jp, i]            (B,   replicated)
    #   ct [ko, kp, o]  = C[cq*256+o, ko*128+kp]     (C.T col-quarter)
    #   xt [ip, io, m]  = x2[rg*8192+m, io*128+ip]   (x row-shard, transposed)
    at_in = nc.dram_tensor("at_in", [KT, P, D], bf16, kind="ExternalInput")
    b_in = nc.dram_tensor("b_in", [KT, P, D], bf16, kind="ExternalInput")
    ct_in = nc.dram_tensor("ct_in", [KT, P, OD], bf16, kind="ExternalInput")
    xt = nc.dram_tensor("xt", [P, KT, MSH], bf16, kind="ExternalInput")
    # y.T shard [o', m]; host transposes + upcasts.
    y_out = nc.dram_tensor("y_out", [OD, MSH], bf16, kind="ExternalOutput")

    with tile.TileContext(nc) as tc:
        with (
            tc.tile_pool(name="big", bufs=1) as big,
            tc.tile_pool(name="ycopy", bufs=6) as ycopy,
            tc.tile_pool(name="ps", bufs=8, space="PSUM") as psp,
        ):
            at_sb = big.tile([P, KT, D], bf16)
            ct_sb = big.tile([P, KT, OD], bf16)
            bt_sb = big.tile([P, KT, D], bf16)
            t_sb = big.tile([P, KT, OD], bf16)
            w_sb = big.tile([P, KT, OD], bf16)
            xt_sb = big.tile([P, KT, MSH], bf16)

            # ---- input DMAs, in exact consumption order ----
            # stage-1 consumes (ct[kt], at[kt]) pairs, kt ascending
            for kt in range(KT):
                nc.sync.dma_start(ct_sb[:, kt, :], ct_in.ap()[kt])
                nc.sync.dma_start(at_sb[:, kt, :], at_in.ap()[kt])
            # stage-2 consumes bt[jt], jt ascending
            for jt in range(KT):
                nc.sync.dma_start(bt_sb[:, jt, :], b_in.ap()[jt])
            # main consumes xt m-chunks, ascending
            for mq in range(8):
                nc.sync.dma_start(
                    xt_sb[:, :, mq * 1024 : (mq + 1) * 1024],
                    xt.ap()[:, :, mq * 1024 : (mq + 1) * 1024],
                )

            # ---- stage 1: T = A @ Ct_q  [1024 x 256], kt-outer ----
            ps1 = [psp.tile([P, MC], f32, name="ps") for j in range(KT)]
            for kt in range(KT):
                for jt in range(KT):
                    nc.tensor.matmul(
                        ps1[jt][:, 0:OD],
                        at_sb[:, kt, jt * P : (jt + 1) * P],
                        ct_sb[:, kt, :],
                        start=(kt == 0),
                        stop=(kt == KT - 1),
                    )
            for jt in range(KT):
                nc.vector.tensor_copy(t_sb[:, jt, :], ps1[jt][:, 0:OD])

            # ---- stage 2: W_q = B.T @ T  [1024 x 256], jt-outer ----
            ps2 = [psp.tile([P, MC], f32, name="ps") for i in range(KT)]
            for jt in range(KT):
                for it in range(KT):
                    nc.tensor.matmul(
                        ps2[it][:, 0:OD],
                        bt_sb[:, jt, it * P : (it + 1) * P],
                        t_sb[:, jt, :],
                        start=(jt == 0),
                        stop=(jt == KT - 1),
                    )
            for it in range(KT):
                nc.vector.tensor_copy(w_sb[:, it, :], ps2[it][:, 0:OD])

            # ---- main: y_q.T = W_q.T @ x.T  [256 x 8192] ----
            # W stationary (reused across m), x moving at N=512.
            # m-groups of 2 chunks x 2 o'-tiles -> 4 psum banks per group.
            for mg in range(NMC // 2):
                pms = [psp.tile([P, MC], f32, name="ps") for i in range(2 * NOT)]
                for ot in range(NOT):
                    for it in range(KT):
                        for mc in range(2):
                            m0 = (mg * 2 + mc) * MC
                            nc.tensor.matmul(
                                pms[2 * ot + mc][:],
                                w_sb[:, it, ot * P : (ot + 1) * P],
                                xt_sb[:, it, m0 : m0 + MC],
                                start=(it == 0),
                                stop=(it == KT - 1),
                            )
                for ot in range(NOT):
                    for mc in range(2):
                        m0 = (mg * 2 + mc) * MC
                        yt = ycopy.tile([P, MC], bf16, name="yt")
                        nc.vector.tensor_copy(yt[:], pms[2 * ot + mc][:])
                        nc.gpsimd.dma_start(
                            y_out.ap()[ot * P : (ot + 1) * P, m0 : m0 + MC],
                            yt[:],
                        )

    nc.compile()
    return nc


def _get_nc():
    if "nc" not in _CACHE:
        _CACHE["nc"] = _build_nc()
    return _CACHE["nc"]


def _make_in_maps(x, A, B, C):
    x2 = np.ascontiguousarray(x, dtype=np.float32).reshape(ROWS, D)
    at = np.ascontiguousarray(np.asarray(A, np.float32).T).reshape(
        KT, P, D
    ).astype(BF16)
    bt = np.asarray(B, np.float32).reshape(KT, P, D).astype(BF16)
    xts = []
    for rg in range(RG):
        shard = x2[rg * MSH : (rg + 1) * MSH]  # [MSH, D]
        xts.append(
            np.ascontiguousarray(
                shard.reshape(MSH, KT, P).transpose(2, 1, 0)
            ).astype(BF16)
        )
    in_maps = []
    for c in range(NCORES):
        rg, cq = divmod(c, CQ)
        csl = np.asarray(C, np.float32)[cq * OD : (cq + 1) * OD, :]  # [OD, D]
        ct = np.ascontiguousarray(csl.T).reshape(KT, P, OD).astype(BF16)
        in_maps.append({"at_in": at, "b_in": bt, "ct_in": ct, "xt": xts[rg]})
    return in_maps


def _install_ntff_hook():
    """The agent image's ``antenv`` lacks ``axon_hooks``; recreate it and
    register the ctypes-based NTFF profile hook (same as trn_boot's
    ``_ntff_profile_via_ctypes``) so ``trace=True`` yields exec_time_ns."""
    import contextlib
    import ctypes
    import types

    if "antenv.axon_hooks" in sys.modules:
        return True
    so_path = "/opt/axon/libaxon_pjrt.so"
    if not os.path.exists(so_path):
        return False
    lib = ctypes.CDLL(so_path)
    if not hasattr(lib, "axon_start_nrt_profile"):
        return False
    lib.axon_start_nrt_profile.argtypes = [
        ctypes.POINTER(ctypes.c_int64),
        ctypes.c_size_t,
    ]
    lib.axon_start_nrt_profile.restype = ctypes.c_int64
    lib.axon_stop_nrt_profile.argtypes = [ctypes.c_char_p]
    lib.axon_stop_nrt_profile.restype = ctypes.c_int64

    @contextlib.contextmanager
    def _hook(output_dir, device_ids):
        import jax

        jax.devices()
        if device_ids:
            ids = (ctypes.c_int64 * len(device_ids))(*device_ids)
            rc = lib.axon_start_nrt_profile(ids, len(device_ids))
        else:
            rc = lib.axon_start_nrt_profile(None, 0)
        if rc != 0:
            raise RuntimeError(f"axon_start_nrt_profile rc={rc}")
        try:
            yield
        finally:
            n = lib.axon_stop_nrt_profile(str(output_dir).encode())
            print(f"ntff profile: {n} file(s) written to {output_dir}")

    mod = types.ModuleType("antenv.axon_hooks")
    _state = {"hook": _hook}
    mod.set_axon_ntff_profile_hook = lambda h: _state.__setitem__("hook", h)
    mod.get_axon_ntff_profile_hook = lambda: _state["hook"]
    sys.modules["antenv.axon_hooks"] = mod
    import antenv

    antenv.axon_hooks = mod
    return True


def run(x, A, B, C, trace=False):
    """Run on hardware; returns (y_full, exec_time_ns_or_None)."""
    from concourse import bass_utils
    from concourse.bass_interp import get_hw_module

    if trace and not _install_ntff_hook():
        trace = False
    if trace:
        # upload_artifacts pushes the NEFF dir to a remote bucket; in this
        # sandbox that can fail AFTER a successful run, losing the results.
        # Degrade to the local path. (Only touches the tracing dev path.)
        if not getattr(bass_utils.upload_artifacts, "_safe", False):
            _orig_upload = bass_utils.upload_artifacts

            def _safe_upload(tmpdir):
                try:
                    return _orig_upload(tmpdir)
                except Exception as e:
                    print(f"upload_artifacts skipped ({type(e).__name__}): {e}")
                    return str(tmpdir)

            _safe_upload._safe = True
            bass_utils.upload_artifacts = _safe_upload

    nc = _get_nc()
    in_maps = _make_in_maps(x, A, B, C)

    old_m = nc.m
    nc.m = get_hw_module(nc.m)
    try:
        res = bass_utils.run_bass_kernel_spmd(
            nc, in_maps, core_ids=list(range(NCORES)), trace=trace
        )
    finally:
        nc.m = old_m

    y2 = np.empty((ROWS, D), dtype=np.float32)
    for c in range(NCORES):
        rg, cq = divmod(c, CQ)
        y2[rg * MSH : (rg + 1) * MSH, cq * OD : (cq + 1) * OD] = (
            res.results[c]["y_out"].T.astype(np.float32)
        )
    return y2.reshape(BATCH, SEQ, D), res.exec_time_ns


def kernel(x, A, B, C):
    y, _ = run(x, A, B, C, trace=False)
    return y
